# revision 32
# baseline (speedup 1.0000x reference)
"""Multi-head causal self-attention with RoPE for Trainium2 (8 NeuronCores).

Problem: B=4, T=2048, C=1024, H=16 heads, D=64, fused QKV + causal softmax
attention + out-projection, fp32 I/O.

Sharding (Megatron-style): core c -> batch b = c//2, heads [8*(c%2), +8).
Each core computes its 8 heads' attention for its batch and a row-parallel
partial of the out-projection; the host sums the two partials per batch.

All matmuls run in bf16 (fp32 PSUM accumulation): bf16 streams the moving
operand at 1 col/cycle @2.4GHz vs ~1.4GHz effective for f32r, and weight
loads get FWL.  Host converts inputs to bf16; output returns as bf16 and is
summed in fp32 on the host.  Tolerance is 2e-2 rel; bf16 lands ~1e-3.

Per-core kernel phases (pipelined over 512-query chunks j):
  1. QKV projections: q^T,k^T in [d,t] layout (head dims permuted into
     even/odd groups of 4 heads for RoPE), v in natural [t,d] layout with a
     ones-column appended (softmax denominators fall out of the attn@V
     matmul for free).  RoPE applied on-chip reading the projection PSUM.
  2. Attention per (chunk j, 4-head group): scores S^T[k,q] via 4-way
     row-packed K=32 matmuls, exp on ScalarE, causal mask multiply,
     attn@V accumulated over key tiles into one 4-bank PSUM accumulator.
     Softmax normalization: per-head one-hot denominator columns in V put
     each head's denominator on its own PSUM partition; a free-axis reduce
     extracts them, a custom-DVE fast reciprocal inverts them, two tiny
     K=4 selector matmuls broadcast them across partitions, and a DVE
     multiply writes the normalized bf16 attention output.
  3. Out-projection per chunk with W_out rows for this core's heads.

PSUM budget (8 banks): "ps2" [128,2,512] x2 bufs (4 banks, shared by
projection pairs / v-proj pairs / score pairs / out-proj pairs) + "av"
[65,4,512] x1 buf (4 banks, the per-head-group attn@V accumulator).
"""

import numpy as np

B, T, C = 4, 2048, 1024
H, D = 16, 64
HC = 8               # heads per core
N_CORES = 8
THETA = 10000.0
NJ = T // 512        # 4 query/column chunks
NKT = T // 128       # 16 key tiles
NCT = C // 128       # 8 contraction tiles for projections

_CACHE = {}


def _build_program(repeat=1):
    """Build the per-core program. repeat>1 replays the whole computation
    (same inputs/outputs) for clean wall-clock timing: the dispatch overhead
    amortizes over `repeat` executions."""
    import concourse.tile as tile
    import concourse.mybir as mybir
    from concourse import bacc

    f32 = mybir.dt.float32
    bf16 = mybir.dt.bfloat16
    EXP = mybir.ActivationFunctionType.Exp
    MUL = mybir.AluOpType.mult
    SUB = mybir.AluOpType.subtract
    ADD = mybir.AluOpType.add

    nc = bacc.Bacc("TRN2", target_bir_lowering=False, debug=False)
    xT_t = nc.dram_tensor("xT", [C, T], bf16, kind="ExternalInput")
    wqk_t = nc.dram_tensor("wqk", [8, 128, NCT, 128], bf16, kind="ExternalInput")
    wv_t = nc.dram_tensor("wv", [C, HC * D], bf16, kind="ExternalInput")
    wout_t = nc.dram_tensor("wout", [HC * D, C], bf16, kind="ExternalInput")
    cs_t = nc.dram_tensor("cs", [128, T], bf16, kind="ExternalInput")
    sn_t = nc.dram_tensor("sn", [128, T], bf16, kind="ExternalInput")
    mk_t = nc.dram_tensor("mk", [128, 1024], bf16, kind="ExternalInput")
    sel_t = nc.dram_tensor("sel", [4, 2, 128], bf16, kind="ExternalInput")
    y_t = nc.dram_tensor("y", [T, C], bf16, kind="ExternalOutput")

    VW = D + 4  # v tile width: 64 dims + 4 one-hot denominator columns

    with tile.TileContext(nc) as tc:
        import contextlib
        with contextlib.ExitStack() as ctx:
            singles = ctx.enter_context(tc.tile_pool(name="singles", bufs=1))
            psum = ctx.enter_context(tc.tile_pool(name="psum", bufs=1, space="PSUM"))
            work = ctx.enter_context(tc.tile_pool(name="work", bufs=1))

            # ---- resident tensors -------------------------------------------
            kT_sb = singles.tile([128, 2, 4, T], bf16, name="kT_sb")
            v_sb = singles.tile([128, 2, NKT, HC, VW], bf16, name="v_sb")
            wqk_sb = singles.tile([128, 8, NCT, 128], bf16, name="wqk_sb")
            wv_sb = singles.tile([128, NCT, HC * D], bf16, name="wv_sb")
            wout_sb = singles.tile([128, 4, C], bf16, name="wout_sb")
            cs_sb = singles.tile([128, T], bf16, name="cs_sb")
            sn_sb = singles.tile([128, T], bf16, name="sn_sb")
            mk_sb = singles.tile([128, 1024], bf16, name="mk_sb")
            sel_sb = singles.tile([4, 2, 128], bf16, name="sel_sb")
            nc.sync.dma_start(sel_sb[:], sel_t.ap())

            nc.sync.dma_start(wqk_sb[:], wqk_t.ap().rearrange("g p k m -> p g k m"))
            nc.sync.dma_start(wv_sb[:], wv_t.ap().rearrange("(kt p) n -> p kt n", p=128))
            nc.sync.dma_start(wout_sb[:], wout_t.ap().rearrange("(ct p) n -> p ct n", p=128))
            nc.sync.dma_start(cs_sb[:], cs_t.ap())
            nc.sync.dma_start(sn_sb[:], sn_t.ap())
            nc.sync.dma_start(mk_sb[:], mk_t.ap())
            # one-hot denominator columns: head h gets an all-ones column at
            # slot D + h%4, so its softmax denominator lands on PSUM
            # partition 64+h%4 of the attn@V accumulator (mk[:, 1023] is all
            # ones).  A free-axis sum over the 4 accumulator banks then
            # extracts all 4 denominators into a 4-partition tile.
            nc.vector.memset(v_sb[:, :, :, :, D:VW], 0.0)
            for par in range(2):
                for h in range(HC):
                    nc.sync.dma_start(
                        v_sb[:, par, :, h, D + h % 4:D + h % 4 + 1],
                        mk_t.ap()[:, None, 1023:1024].broadcast_to([128, NKT, 1]))

            X = mybir.AxisListType.X

            def emit_xtn(rep, j):
                xtn = []
                c0 = 512 * j
                for k in range(NCT):
                    t_ = work.tile([128, 512], bf16, tag="xtn", bufs=16,
                                   name=f"r{rep}_xtn{j}_{k}")
                    nc.sync.dma_start(t_[:], xT_t.ap()[128 * k:128 * k + 128,
                                                       c0:c0 + 512])
                    xtn.append(t_)
                return xtn

            def emit_proj_qk(rep, j, gps, xtn, qTc):
                par = rep % 2
                # (even, odd) projection pairs land in one 2-bank PSUM tile;
                # RoPE reads the PSUM directly and writes rotated bf16
                # q^T/k^T straight to SBUF.
                c0 = 512 * j
                for gp in gps:
                    ps = psum.tile([128, 2, 512], f32, tag="ps2", bufs=2,
                                   name=f"r{rep}_pqk{j}_{gp}")
                    for t in range(2):
                        g = gp + t
                        for k in range(NCT):
                            nc.tensor.matmul(ps[:, t, :], wqk_sb[:, g, k, :],
                                             xtn[k][:],
                                             start=(k == 0), stop=(k == NCT - 1))
                    pe = ps[:, 0, :]
                    po = ps[:, 1, :]
                    css = cs_sb[:, c0:c0 + 512]
                    sns = sn_sb[:, c0:c0 + 512]
                    if gp < 4:
                        x1 = qTc[:, gp, :]
                        x2 = qTc[:, gp + 1, :]
                    else:
                        x1 = kT_sb[:, par, gp - 4, c0:c0 + 512]
                        x2 = kT_sb[:, par, gp - 3, c0:c0 + 512]
                    t1 = work.tile([128, 512], bf16, tag="rt", bufs=8,
                                   name=f"r{rep}_t1_{j}_{gp}")
                    nc.vector.tensor_tensor(t1[:], pe, css, MUL)
                    t2 = work.tile([128, 512], bf16, tag="rt", bufs=8,
                                   name=f"r{rep}_t2_{j}_{gp}")
                    nc.vector.tensor_tensor(t2[:], pe, sns, MUL)
                    t3 = work.tile([128, 512], bf16, tag="rt", bufs=8,
                                   name=f"r{rep}_t3_{j}_{gp}")
                    nc.vector.tensor_tensor(t3[:], po, sns, MUL)
                    nc.vector.tensor_tensor(x1, t1[:], t3[:], SUB)
                    t4 = work.tile([128, 512], bf16, tag="rt", bufs=8,
                                   name=f"r{rep}_t4_{j}_{gp}")
                    nc.vector.tensor_tensor(t4[:], po, css, MUL)
                    nc.vector.tensor_tensor(x2, t4[:], t2[:], ADD)

            def emit_proj_v(rep, j, xtn):
                par = rep % 2
                for tp in range(2):
                    ps = psum.tile([128, 2, 512], f32, tag="ps2", bufs=2,
                                   name=f"r{rep}_pv{j}_{tp}")
                    for tt in range(2):
                        t4i = 2 * tp + tt
                        for k in range(NCT):
                            nc.tensor.matmul(ps[:, tt, :],
                                             xtn[k][:, 128 * t4i:128 * t4i + 128],
                                             wv_sb[:, k, :],
                                             start=(k == 0), stop=(k == NCT - 1))
                    kt0 = 4 * j + 2 * tp
                    nc.vector.tensor_copy(
                        v_sb[:, par, kt0:kt0 + 2, :, 0:D],
                        ps[:].rearrange("p a (h d) -> p a h d", h=HC))

            def emit_attn_half(rep, j, hg, qTc):
                par = rep % 2
                ge, go = 2 * hg, 2 * hg + 1
                nk = 4 * (j + 1)
                av = psum.tile([D + 4, 4, 512], f32, tag="av", bufs=1,
                               name=f"r{rep}_av{j}_{hg}")
                for i in range(nk):
                    # diagonal tiles: columns [0, off) are fully masked —
                    # skip them; apply the 128-wide triangular mask at
                    # [off, off+128).
                    off = 128 * i - 512 * j
                    lo = max(off, 0)
                    for hp in range(2):  # head pairs: 2 PSUM banks/exp
                        sps = psum.tile([128, 2, 512], f32, tag="ps2", bufs=2,
                                        name=f"r{rep}_sps{j}_{hg}_{i}_{hp}")
                        for sub in range(2):
                            h4 = 2 * hp + sub
                            r0 = 32 * h4
                            tp_ = (r0, 0)
                            nc.tensor.matmul(
                                sps[:, sub, lo:512],
                                kT_sb[r0:r0 + 32, par, ge, 128 * i:128 * i + 128],
                                qTc[r0:r0 + 32, ge, lo:512],
                                start=True, stop=False, tile_position=tp_)
                            nc.tensor.matmul(
                                sps[:, sub, lo:512],
                                kT_sb[r0:r0 + 32, par, go, 128 * i:128 * i + 128],
                                qTc[r0:r0 + 32, go, lo:512],
                                start=False, stop=True, tile_position=tp_)
                        pt = work.tile([128, 2, 512], bf16, tag="pt", bufs=6,
                                       name=f"r{rep}_pt{j}_{hg}_{i}_{hp}")
                        nc.scalar.activation(pt[:, :, lo:512],
                                             sps[:, :, lo:512], EXP)
                        if off >= 0:
                            nc.vector.tensor_tensor(
                                pt[:, :, off:off + 128],
                                pt[:, :, off:off + 128],
                                mk_sb[:, None, 512:640]
                                .broadcast_to([128, 2, 128]), MUL)
                        for sub in range(2):
                            h4 = 2 * hp + sub
                            h = 4 * hg + h4
                            nc.tensor.matmul(
                                av[:, h4, lo:512],
                                v_sb[:, par, i, h, :], pt[:, sub, lo:512],
                                start=(i == 0), stop=(i == nk - 1))
                return av

            def emit_drain(rep, j, hg, av, aot):
                # normalization: denominators live one per partition on rows
                # 64..67 (one-hot column trick); the off-diagonal slots are
                # zero, so a free-axis sum extracts all 4 into [4, 512],
                # then one custom-DVE reciprocal + per-head broadcast.
                dred = work.tile([4, 512], f32, tag="dred", bufs=2,
                                 name=f"r{rep}_dr{j}_{hg}")
                nc.vector.tensor_reduce(
                    dred[:], av[D:D + 4, :, :].rearrange("p a q -> p q a"),
                    X, ADD)
                rc = work.tile([4, 512], f32, tag="rc", bufs=2,
                               name=f"r{rep}_rc{j}_{hg}")
                nc.vector.reciprocal_approx_fast(rc[:], dred[:])
                rcb = work.tile([4, 512], bf16, tag="rcb", bufs=2,
                                name=f"r{rep}_rcb{j}_{hg}")
                nc.vector.tensor_scalar_mul(rcb[:], rc[:], 1.0)
                # broadcast 1/den across the 64 partitions of each head's
                # aot slice with two tiny K=4 selector matmuls on the PE
                bcps = psum.tile([128, 2, 512], f32, tag="ps2", bufs=2,
                                 name=f"r{rep}_bc{j}_{hg}")
                for c in range(2):
                    nc.tensor.matmul(bcps[:, c, :], sel_sb[:, c, :], rcb[:],
                                     start=True, stop=True)
                bcs = work.tile([128, 2, 512], f32, tag="bcs", bufs=2,
                                name=f"r{rep}_bcs{j}_{hg}")
                nc.vector.tensor_copy(bcs[:], bcps[:])
                for h4 in range(4):
                    h = 4 * hg + h4
                    nc.vector.tensor_tensor(
                        aot[64 * (h % 2):64 * (h % 2) + 64, h // 2, :],
                        av[0:D, h4, :],
                        bcs[64 * (h4 % 2):64 * (h4 % 2) + 64, h4 // 2, :],
                        MUL)

            def emit_outproj(rep, j, aot):
                c0 = 512 * j
                for tt4 in range(4):
                    ps = psum.tile([128, 2, 512], f32, tag="ps2", bufs=2,
                                   name=f"r{rep}_yps{j}_{tt4}")
                    for cc in range(2):
                        for ct in range(4):
                            nc.tensor.matmul(
                                ps[:, cc, :],
                                aot[:, ct, 128 * tt4:128 * tt4 + 128],
                                wout_sb[:, ct, 512 * cc:512 * cc + 512],
                                start=(ct == 0), stop=(ct == 3))
                    yst = work.tile([128, 2, 512], bf16, tag="yst", bufs=3,
                                    name=f"r{rep}_yst{j}_{tt4}")
                    nc.vector.tensor_copy(yst[:], ps[:])
                    nc.sync.dma_start(
                        y_t.ap()[c0 + 128 * tt4:c0 + 128 * tt4 + 128, :]
                        .rearrange("r (a c) -> r a c", a=2),
                        yst[:])

            # ---- software-pipelined main loop: a flat sequence of
            # (rep, chunk) elements; during the attention / normalization of
            # the previous chunk, the PE queue holds the current chunk's
            # projection matmuls so the drains never idle the PE (keeps HAM
            # at full clock).  kT/v are double-buffered by rep parity so the
            # pipeline also flows across rep boundaries.
            chunks = [(rep, j) for rep in range(repeat) for j in range(NJ)]
            prev = None
            for rep, jj in chunks:
                xtn = emit_xtn(rep, jj)
                qTc = work.tile([128, 4, 512], bf16, tag="qTc", bufs=2,
                                name=f"r{rep}_qTc{jj}")
                aot = work.tile([128, 4, 512], bf16, tag="aot", bufs=2,
                                name=f"r{rep}_aot{jj}")
                if prev:
                    av0 = emit_attn_half(prev[0], prev[1], 0, prev[2])
                emit_proj_qk(rep, jj, (0,), xtn, qTc)
                if prev:
                    emit_drain(prev[0], prev[1], 0, av0, prev[3])
                emit_proj_qk(rep, jj, (2,), xtn, qTc)
                if prev:
                    av1 = emit_attn_half(prev[0], prev[1], 1, prev[2])
                emit_proj_qk(rep, jj, (4,), xtn, qTc)
                if prev:
                    emit_drain(prev[0], prev[1], 1, av1, prev[3])
                emit_proj_qk(rep, jj, (6,), xtn, qTc)
                emit_proj_v(rep, jj, xtn)
                if prev:
                    emit_outproj(prev[0], prev[1], prev[3])
                prev = (rep, jj, qTc, aot)
            av0 = emit_attn_half(prev[0], prev[1], 0, prev[2])
            emit_drain(prev[0], prev[1], 0, av0, prev[3])
            av1 = emit_attn_half(prev[0], prev[1], 1, prev[2])
            emit_drain(prev[0], prev[1], 1, av1, prev[3])
            emit_outproj(prev[0], prev[1], prev[3])

    nc.compile()
    return nc


def _host_inputs(x, W_qkv, W_out):
    """Per-core input dicts (numpy, bf16)."""
    import ml_dtypes
    bf16 = ml_dtypes.bfloat16
    x = np.ascontiguousarray(np.asarray(x), dtype=np.float32)
    W_qkv = np.ascontiguousarray(np.asarray(W_qkv), dtype=np.float32)
    W_out = np.ascontiguousarray(np.asarray(W_out), dtype=np.float32)

    inv_freq = (1.0 / (THETA ** (np.arange(0, D, 2, dtype=np.float32) / D))).astype(np.float32)
    freqs = np.arange(T, dtype=np.float32)[:, None] * inv_freq[None, :]  # [T, 32]
    cs = np.tile(np.cos(freqs).T.astype(np.float32), (4, 1)).astype(bf16)  # [128, T]
    sn = np.tile(np.sin(freqs).T.astype(np.float32), (4, 1)).astype(bf16)
    kk = np.arange(128)[:, None]
    cc = np.arange(1024)[None, :]
    mk = (cc >= kk + 512).astype(bf16)
    # selector patterns for the 1/den partition-broadcast matmuls:
    # sel[2c+par, c, 64*par:+64] = 1
    sel = np.zeros((4, 2, 128), bf16)
    for c_ in range(2):
        for par in range(2):
            sel[2 * c_ + par, c_, 64 * par:64 * par + 64] = 1.0

    in_maps = []
    for core in range(N_CORES):
        b, hg = core // 2, core % 2
        h0 = HC * hg  # first global head
        # permuted q/k columns: groups of 128 = (4 heads) x (32 even-or-odd dims)
        cols = []
        for s in range(2):  # 0=q, 1=k
            for quad in range(2):          # heads [4*quad, 4*quad+4)
                for par in range(2):       # 0=even dims, 1=odd dims
                    for hh in range(4):
                        hglob = h0 + 4 * quad + hh
                        for i_ in range(32):
                            cols.append(s * (H * D) + hglob * D + 2 * i_ + par)
        cols = np.asarray(cols)
        wqk = W_qkv[:, cols].copy()
        wqk[:, 0:512] *= np.float32(1.0 / np.sqrt(D))  # fold score scale into Wq
        # pre-tile to the SBUF layout: [group, partition, ktile, m]
        wqk = np.ascontiguousarray(
            wqk.reshape(NCT, 128, 8, 128).transpose(2, 1, 0, 3)).astype(bf16)
        wv = W_qkv[:, 2 * H * D + h0 * D: 2 * H * D + (h0 + HC) * D].astype(bf16)
        wout = W_out[h0 * D:(h0 + HC) * D, :].astype(bf16)
        in_maps.append({
            "xT": np.ascontiguousarray(x[b].T).astype(bf16),
            "wqk": wqk, "wv": wv, "wout": wout,
            "cs": cs, "sn": sn, "mk": mk, "sel": sel,
        })
    return in_maps


def _get_runtime(repeat=1):
    """Compile once; return a cached sharded jitted callable + metadata."""
    key = ("rt", repeat)
    if key in _CACHE:
        return _CACHE[key]
    import jax
    import numpy as _np
    from jax.sharding import Mesh, PartitionSpec
    from jax.experimental.shard_map import shard_map
    import concourse.mybir as mybir
    from concourse import bass2jax

    nc = _build_program(repeat=repeat)
    bass2jax.install_neuronx_cc_hook()

    partition_name = (nc.partition_id_tensor.name
                      if nc.partition_id_tensor else None)
    in_names, out_names, out_avals, zero_outs = [], [], [], []
    for alloc in nc.m.functions[0].allocations:
        if not isinstance(mybir_alloc := alloc, mybir.MemoryLocationSet):
            continue
        name = alloc.memorylocations[0].name
        if alloc.kind == "ExternalInput":
            if name != partition_name:
                in_names.append(name)
        elif alloc.kind == "ExternalOutput":
            np_dt = mybir.dt.np(alloc.dtype)
            out_names.append(name)
            out_avals.append(jax.core.ShapedArray(tuple(alloc.tensor_shape), np_dt))
            zero_outs.append(_np.zeros(tuple(alloc.tensor_shape), np_dt))

    n_params = len(in_names)
    n_outs = len(out_names)
    all_in_names = list(in_names) + list(out_names)
    if partition_name is not None:
        all_in_names.append(partition_name)
    donate = tuple(range(n_params, n_params + n_outs))

    def _body(*args):
        operands = list(args)
        if partition_name is not None:
            operands.append(bass2jax.partition_id_tensor())
        outs = bass2jax._bass_exec_p.bind(
            *operands,
            out_avals=tuple(out_avals),
            in_names=tuple(all_in_names),
            out_names=tuple(out_names),
            lowering_input_output_aliases=(),
            sim_require_finite=True,
            sim_require_nnan=True,
            nc=nc,
        )
        return tuple(outs)

    devices = jax.devices()[:N_CORES]
    mesh = Mesh(np.asarray(devices), ("core",))
    in_specs = (PartitionSpec("core"),) * (n_params + n_outs)
    out_specs = (PartitionSpec("core"),) * n_outs
    fn = jax.jit(
        shard_map(_body, mesh=mesh, in_specs=in_specs, out_specs=out_specs,
                  check_rep=False),
        donate_argnums=donate, keep_unused=True)

    rt = dict(fn=fn, in_names=in_names, out_names=out_names,
              zero_outs=zero_outs, mesh=mesh)
    _CACHE[key] = rt
    return rt


def _run(in_maps):
    rt = _get_runtime()
    concat_in = [np.concatenate([np.asarray(in_maps[c][n]) for c in range(N_CORES)],
                                axis=0) for n in rt["in_names"]]
    concat_zeros = [np.zeros((N_CORES * z.shape[0], *z.shape[1:]), z.dtype)
                    for z in rt["zero_outs"]]
    out_arrs = rt["fn"](*concat_in, *concat_zeros)
    (y_name,) = rt["out_names"]
    y_all = np.asarray(out_arrs[0]).reshape(N_CORES, T, C)
    return y_all


def kernel(x, W_qkv, W_out):
    in_maps = _host_inputs(x, W_qkv, W_out)
    y_all = _run(in_maps)
    y = np.empty((B, T, C), dtype=np.float32)
    for b in range(B):
        y[b] = y_all[2 * b].astype(np.float32) + y_all[2 * b + 1].astype(np.float32)
    return y
